# revision 1
# baseline (speedup 1.0000x reference)
"""GATv2 layer kernel for Trainium2 (8 NeuronCores, SPMD).

Math note: in the reference, the per-edge value vectors are gathered from the
*destination* node (Vv = V[dest] @ Wv^T + bv) and the scatter-softmax is also
grouped by destination. Within a destination segment Vv is constant, and the
softmax weights sum to 1, so

    H[n] = (V[n] @ Wv_w^T + Wv_b) * [n has >= 1 incoming edge]

exactly (up to f32 rounding of the softmax-weight sum, ~1e-7 relative).

Sharding: nodes are partitioned contiguously across the 8 cores; edges are
dest-partitioned so the per-node "has incoming edge" reduction stays local to
the core that owns the node (no collectives). The small [128,128] weights are
replicated. Each core computes the Wv projection of its node shard with the
tensor engine and, when needed, derives the incoming-edge mask on-device via
a GPSIMD SWDGE scatter-add histogram over its local edge destinations.

If every node has at least one incoming edge (checked on host; true with
overwhelming probability at E/N = 12.5), the mask multiply is the identity and
a maskless variant is dispatched.
"""

import numpy as np

import concourse.bacc as bacc
import concourse.bass as bass
import concourse.mybir as mybir
import concourse.tile as tile
from concourse.bass_utils import run_bass_kernel_spmd
from concourse.library_config import mlp

N_CORES = 8
P = 128
D = 128
TABLE_W = 64  # f32 words per histogram-table row -> 256B stride (SWDGE req.)

_module_cache = {}

# Cap indices per SWDGE scatter-add: the Q7 expands indices to int32 in
# local scratch (4096 validated on HW; 8192 crashes the exec unit).
MAX_IDXS_PER_SCATTER = 4096


def _chunking(pad_idx):
    n_chunks = -(-pad_idx // MAX_IDXS_PER_SCATTER)
    per_chunk = -(-pad_idx // (n_chunks * P)) * P
    return n_chunks, per_chunk


def _build_module(n_tiles, pad_idx):
    """One SPMD NeuronCore program: h = (v @ wvT + b) * mask.

    n_tiles: 128-row node tiles per core (v/h are [n_tiles*128, 128]).
    pad_idx: padded per-core edge count for the mask histogram (multiple of
        128), or None for the maskless variant.
    """
    f32 = mybir.dt.float32
    NP = n_tiles * P
    masked = pad_idx is not None

    nc = bacc.Bacc("TRN2", target_bir_lowering=False, debug=False)
    # node features arrive transposed ([D, nodes]) so tiles feed the PE's
    # lhsT port directly (contraction dim on partitions), full-line DMA
    vT_in = nc.dram_tensor("vT", [D, NP], f32, kind="ExternalInput")
    wvT_in = nc.dram_tensor("wvT", [D, D], f32, kind="ExternalInput")
    b_in = nc.dram_tensor("b", [1, D], f32, kind="ExternalInput")
    h_out = nc.dram_tensor("h", [NP, D], f32, kind="ExternalOutput")
    if masked:
        # num_idxs is a uint16 ISA field: split the histogram into chunks.
        n_chunks, per_chunk = _chunking(pad_idx)
        cols = per_chunk // 16
        idxs_in = nc.dram_tensor(
            "idxs", [P, n_chunks, cols], mybir.dt.int16, kind="ExternalInput"
        )
        # ExternalOutput: the runtime hands the kernel a pre-zeroed buffer,
        # which the scatter-add then accumulates into.
        table_out = nc.dram_tensor("tbl", [NP, TABLE_W], f32, kind="ExternalOutput")

    # pipeline granularity: groups of node tiles so loads/compute/stores overlap
    import os
    group = int(os.environ.get("K_GROUP", "4"))
    vbufs = int(os.environ.get("K_VBUFS", "3"))
    hbufs = int(os.environ.get("K_HBUFS", "3"))
    psbufs = int(os.environ.get("K_PSBUFS", "6"))
    repeat = int(os.environ.get("K_REPEAT", "1"))  # timing experiments only
    n_groups = -(-n_tiles // group)

    with tile.TileContext(nc) as tc:
        with (
            tc.tile_pool(name="const", bufs=1) as cpool,
            tc.tile_pool(name="vg", bufs=vbufs) as vpool,
            tc.tile_pool(name="hg", bufs=hbufs) as hpool,
            tc.tile_pool(name="psh", bufs=psbufs, space="PSUM") as pspool_h,
        ):
            mask_sb = None
            if masked:
                nc.gpsimd.load_library(mlp)
                idxs_sb = cpool.tile([P, n_chunks, cols], mybir.dt.int16)
                nc.sync.dma_start(out=idxs_sb[:], in_=idxs_in[:])
                ones_src = cpool.tile([P, per_chunk // P, 1], f32)
                nc.gpsimd.memset(ones_src[:], 1.0)
                # The SWDGE scatter-add ISA struct cannot carry sync waits;
                # absorb the idxs-DMA dependency on a cheap gpsimd op first.
                dep_sink = cpool.tile([P, 8], mybir.dt.int16)
                nc.gpsimd.tensor_copy(out=dep_sink[:], in_=idxs_sb[:, 0, :8])
                for ch in range(n_chunks):
                    nc.gpsimd.dma_scatter_add(
                        table_out[:, 0:1],
                        ones_src[:],
                        idxs_sb[:, ch, :],
                        per_chunk,
                        per_chunk,
                        1,
                        elem_step=TABLE_W,
                    )
                tblr_sb = cpool.tile([P, n_tiles * TABLE_W], f32)
                nc.sync.dma_start(
                    out=tblr_sb[:].rearrange("p (t e) -> p t e", e=TABLE_W),
                    in_=table_out[:].rearrange("(p t) e -> p t e", p=P),
                )
                mask_sb = cpool.tile([P, n_tiles], f32)
                counts_view = tblr_sb[:].rearrange(
                    "p (t e) -> p t e", e=TABLE_W
                )[:, :, 0:1]
                nc.vector.tensor_scalar(
                    out=mask_sb[:],
                    in0=counts_view,
                    scalar1=0.0,
                    scalar2=None,
                    op0=mybir.AluOpType.is_gt,
                )

            wvT_sb = cpool.tile([D, D], f32)
            nc.sync.dma_start(out=wvT_sb[:], in_=wvT_in[:])
            b_sb = cpool.tile([1, D], f32)
            nc.sync.dma_start(out=b_sb[:], in_=b_in[:])
            ones_row = cpool.tile([1, P], f32)
            nc.vector.memset(ones_row[:], 1.0)

            for g in range(n_groups * repeat):
                g = g % n_groups
                t0 = g * group
                gt = min(group, n_tiles - t0)
                v_sb = vpool.tile([P, group * D], f32, tag="vg")
                nc.sync.dma_start(
                    out=v_sb[:, : gt * D], in_=vT_in[:, t0 * D : (t0 + gt) * D]
                )
                h_sb = hpool.tile([P, group * D], f32, tag="hg")
                for i in range(gt):
                    t = t0 + i
                    h_ps = pspool_h.tile([P, P], f32, tag="hps")
                    nc.tensor.matmul(
                        out=h_ps[:],
                        lhsT=v_sb[:, i * P : (i + 1) * P],
                        rhs=wvT_sb[:],
                        start=True,
                        stop=False,
                    )
                    nc.tensor.matmul(
                        out=h_ps[:], lhsT=ones_row[:], rhs=b_sb[:],
                        start=False, stop=True,
                    )
                    if masked:
                        nc.vector.tensor_scalar_mul(
                            h_sb[:, i * D : (i + 1) * D], h_ps[:],
                            mask_sb[:, t : t + 1],
                        )
                    else:
                        nc.vector.tensor_copy(
                            out=h_sb[:, i * D : (i + 1) * D], in_=h_ps[:]
                        )
                nc.sync.dma_start(
                    out=h_out[t0 * P : (t0 + gt) * P, :].rearrange(
                        "(g p) d -> p g d", p=P
                    ),
                    in_=h_sb[:, : gt * D].rearrange("p (g d) -> p g d", d=D),
                )

    nc.compile()
    return nc


def _get_module(n_tiles, pad_idx):
    key = (n_tiles, pad_idx)
    if key not in _module_cache:
        _module_cache[key] = _build_module(n_tiles, pad_idx)
    return _module_cache[key]


def kernel(V, E, edge_index, Wq_w, Wq_b, Wk_w, Wk_b, Wv_w, Wv_b, We_w, We_b,
           a_w, a_b, _trace=False):
    V = np.ascontiguousarray(np.asarray(V, dtype=np.float32))
    n_nodes, d = V.shape
    assert d == D and n_nodes % N_CORES == 0
    npc = n_nodes // N_CORES          # nodes per core
    n_tiles = -(-npc // P)            # 128-row tiles per core
    NP = n_tiles * P

    dest = np.asarray(edge_index)[1].astype(np.int64)
    counts = np.bincount(dest, minlength=n_nodes)
    covered = bool(counts.min() > 0)

    wvT = np.ascontiguousarray(np.asarray(Wv_w, dtype=np.float32).T)
    brow = np.ascontiguousarray(np.asarray(Wv_b, dtype=np.float32)[None, :])

    in_maps = []
    pad_idx = None
    if not covered:
        # dest-partition the edges; per-core local histogram indices,
        # permuted to the table layout row = (n%128)*n_tiles + n//128.
        core_of = dest // npc
        locs = []
        for c in range(N_CORES):
            n_loc = dest[core_of == c] - c * npc
            if len(n_loc) > 20 * MAX_IDXS_PER_SCATTER:
                # beyond the HW-validated per-core scatter envelope (extreme
                # dest skew): scatter only the distinct local nodes instead
                n_loc = np.unique(n_loc)
            locs.append(((n_loc % P) * n_tiles + n_loc // P).astype(np.int16))
        max_cnt = max(len(x) for x in locs)
        pad_idx = -(-max_cnt // P) * P
        n_chunks, per_chunk = _chunking(pad_idx)
        cols = per_chunk // 16

    for c in range(N_CORES):
        vpT = np.zeros((D, NP), dtype=np.float32)
        vpT[:, :npc] = V[c * npc : (c + 1) * npc].T
        m = {"vT": vpT, "wvT": wvT, "b": brow}
        if not covered:
            # real indices first, then trailing -1 pads; chunked so pads are
            # trailing within each chunk (the SWDGE trims trailing negatives)
            flat = np.full(n_chunks * per_chunk, -1, dtype=np.int16)
            flat[: len(locs[c])] = locs[c]
            chunks = [
                np.tile(np.ascontiguousarray(ck.reshape(cols, 16).T), (N_CORES, 1))
                for ck in flat.reshape(n_chunks, per_chunk)
            ]
            m["idxs"] = np.ascontiguousarray(np.stack(chunks, axis=1))
        in_maps.append(m)

    nc = _get_module(n_tiles, pad_idx)
    res = run_bass_kernel_spmd(nc, in_maps, core_ids=list(range(N_CORES)),
                               trace=_trace)
    out = np.concatenate([res.results[c]["h"][:npc] for c in range(N_CORES)], axis=0)
    if _trace:
        return out, res
    return out



# revision 2
# speedup vs baseline: 2.9619x; 2.9619x over previous
"""GATv2 layer kernel for Trainium2 (8 NeuronCores, SPMD).

Math note: in the reference, the per-edge value vectors are gathered from the
*destination* node (Vv = V[dest] @ Wv^T + bv) and the scatter-softmax is also
grouped by destination. Within a destination segment Vv is constant, and the
softmax weights sum to 1, so

    H[n] = (V[n] @ Wv_w^T + Wv_b) * [n has >= 1 incoming edge]

exactly (up to f32 rounding of the softmax-weight sum, ~1e-7 relative).

Device kernel: per-core masked-matmul in transposed layout. Nodes are
partitioned contiguously across the 8 cores; the small [128,128] weight is
replicated and kept stationary in the PE array, while node features stream
through as the moving operand in 512-wide slices (PSUM bank width). The
PSUM->SBUF drain on the vector engine fuses the per-partition bias add and
the bf16 downcast. bf16 halves HBM traffic and runs the PE at full rate;
the rel-err budget (2e-2) dwarfs bf16 rounding (~3e-3).

Layout: compute h^T = Wv @ v^T so the weight is stationary; v^T/h^T live in
DRAM as [128, nodes] with multi-KB contiguous partition lines (efficient
DMA). Host stages the transposes/casts and zeroes the (rare) nodes with no
incoming edge after the gather.
"""

import os

import ml_dtypes
import numpy as np

import concourse.bacc as bacc
import concourse.bass as bass
import concourse.mybir as mybir
import concourse.tile as tile
from concourse.bass_utils import run_bass_kernel_spmd

N_CORES = 8
P = 128
D = 128
F = 512  # matmul moving-operand free size = one PSUM bank of f32

_module_cache = {}


def _build_module(n_slices):
    """One SPMD NeuronCore program: hT = wvT.T @ vT + b (bf16 I/O).

    n_slices: number of 512-column node slices per core
    (vT/hT are [128, n_slices*512] bf16).
    """
    f32 = mybir.dt.float32
    bf16 = mybir.dt.bfloat16
    NP = n_slices * F

    nc = bacc.Bacc("TRN2", target_bir_lowering=False, debug=False)
    vT_in = nc.dram_tensor("vT", [D, NP], bf16, kind="ExternalInput")
    wvT_in = nc.dram_tensor("wvT", [D, D], bf16, kind="ExternalInput")
    b_in = nc.dram_tensor("b", [D, 1], f32, kind="ExternalInput")
    hT_out = nc.dram_tensor("hT", [D, NP], bf16, kind="ExternalOutput")

    group = int(os.environ.get("K_GROUP", "2"))  # 512-col slices per DMA chunk
    vbufs = int(os.environ.get("K_VBUFS", "3"))
    hbufs = int(os.environ.get("K_HBUFS", "3"))
    psbufs = int(os.environ.get("K_PSBUFS", "8"))
    n_groups = -(-n_slices // group)

    with tile.TileContext(nc) as tc:
        with (
            tc.tile_pool(name="const", bufs=1) as cpool,
            tc.tile_pool(name="vg", bufs=vbufs) as vpool,
            tc.tile_pool(name="hg", bufs=hbufs) as hpool,
            tc.tile_pool(name="psh", bufs=psbufs, space="PSUM") as pspool,
        ):
            wvT_sb = cpool.tile([D, D], bf16)
            nc.sync.dma_start(out=wvT_sb[:], in_=wvT_in[:])
            b_sb = cpool.tile([D, 1], f32)
            nc.sync.dma_start(out=b_sb[:], in_=b_in[:])

            for g in range(n_groups):
                s0 = g * group
                gs = min(group, n_slices - s0)
                v_sb = vpool.tile([D, group * F], bf16, tag="vg")
                nc.sync.dma_start(
                    out=v_sb[:, : gs * F], in_=vT_in[:, s0 * F : (s0 + gs) * F]
                )
                h_sb = hpool.tile([D, group * F], bf16, tag="hg")
                for i in range(gs):
                    h_ps = pspool.tile([P, F], f32, tag="hps")
                    nc.tensor.matmul(
                        out=h_ps[:],
                        lhsT=wvT_sb[:],
                        rhs=v_sb[:, i * F : (i + 1) * F],
                        start=True,
                        stop=True,
                    )
                    nc.vector.tensor_scalar_add(
                        h_sb[:, i * F : (i + 1) * F], h_ps[:], b_sb[:]
                    )
                nc.sync.dma_start(
                    out=hT_out[:, s0 * F : (s0 + gs) * F], in_=h_sb[:, : gs * F]
                )

    nc.compile()
    return nc


def _get_module(n_slices):
    if n_slices not in _module_cache:
        _module_cache[n_slices] = _build_module(n_slices)
    return _module_cache[n_slices]


def kernel(V, E, edge_index, Wq_w, Wq_b, Wk_w, Wk_b, Wv_w, Wv_b, We_w, We_b,
           a_w, a_b, _trace=False):
    V = np.asarray(V, dtype=np.float32)
    n_nodes, d = V.shape
    assert d == D and n_nodes % N_CORES == 0
    npc = n_nodes // N_CORES          # nodes per core
    n_slices = -(-npc // F)           # 512-col slices per core
    NP = n_slices * F

    bf16 = ml_dtypes.bfloat16
    wvT = np.ascontiguousarray(np.asarray(Wv_w, dtype=np.float32).T.astype(bf16))
    bcol = np.ascontiguousarray(np.asarray(Wv_b, dtype=np.float32)[:, None])

    in_maps = []
    for c in range(N_CORES):
        vpT = np.zeros((D, NP), dtype=bf16)
        vpT[:, :npc] = V[c * npc : (c + 1) * npc].astype(bf16).T
        in_maps.append({"vT": vpT, "wvT": wvT, "b": bcol})

    nc = _get_module(n_slices)
    res = run_bass_kernel_spmd(nc, in_maps, core_ids=list(range(N_CORES)),
                               trace=_trace)
    out = np.concatenate(
        [np.asarray(res.results[c]["hT"])[:, :npc].T.astype(np.float32)
         for c in range(N_CORES)],
        axis=0,
    )

    # nodes with no incoming edge have an empty softmax segment -> H row = 0
    dest = np.asarray(edge_index)[1]
    counts = np.bincount(dest.astype(np.int64), minlength=n_nodes)
    uncovered = np.flatnonzero(counts == 0)
    if uncovered.size:
        out[uncovered] = 0.0

    if _trace:
        return out, res
    return out


# revision 3
# speedup vs baseline: 3.1129x; 1.0510x over previous
"""GATv2 layer kernel for Trainium2 (8 NeuronCores, SPMD).

Math note: in the reference, the per-edge value vectors are gathered from the
*destination* node (Vv = V[dest] @ Wv^T + bv) and the scatter-softmax is also
grouped by destination. Within a destination segment Vv is constant, and the
softmax weights sum to 1, so

    H[n] = (V[n] @ Wv_w^T + Wv_b) * [n has >= 1 incoming edge]

exactly (up to f32 rounding of the softmax-weight sum, ~1e-7 relative).

Device kernel: per-core matmul in transposed layout, h^T = Wv @ v^T. Nodes
are partitioned contiguously across the 8 cores; the small [128,128] weight
is replicated and kept stationary in the PE array while node features stream
through as the moving operand in 512-wide slices (PSUM bank width). The
PSUM->SBUF drain fuses the per-partition bias add and the bf16 downcast,
alternating between the vector and scalar engines so neither becomes the
pipeline bottleneck. bf16 I/O halves HBM traffic and runs the PE at full
rate; the rel-err budget (2e-2) dwarfs bf16 rounding (~3e-3).

DMA layout: v^T/h^T live in DRAM as [128, nodes] so partition lines are
multi-KB and contiguous. All input chunks are prefetched up front on the
sync engine's HWDGE queue; output chunks go out on the same queue after all
input triggers (emission order keeps the in-order engine from stalling
loads behind stores). Host stages the transposes/casts and zeroes the
(rare) nodes with no incoming edge after the gather.
"""

import os

import ml_dtypes
import numpy as np

import concourse.bacc as bacc
import concourse.bass as bass
import concourse.mybir as mybir
import concourse.tile as tile
from concourse.bass_utils import run_bass_kernel_spmd

N_CORES = 8
P = 128
D = 128
F = 512  # matmul moving-operand free size = one PSUM bank of f32

_module_cache = {}


def _build_module(n_tiles):
    """One SPMD NeuronCore program: hT = wvT.T @ vT + b (bf16 I/O).

    n_tiles: 128-column node tiles per core (vT/hT are [128, n_tiles*128]).
    """
    f32 = mybir.dt.float32
    bf16 = mybir.dt.bfloat16
    NP = n_tiles * P

    nc = bacc.Bacc("TRN2", target_bir_lowering=False, debug=False)
    vT_in = nc.dram_tensor("vT", [D, NP], bf16, kind="ExternalInput")
    wvT_in = nc.dram_tensor("wvT", [D, D], bf16, kind="ExternalInput")
    b_in = nc.dram_tensor("b", [D, 1], f32, kind="ExternalInput")
    hT_out = nc.dram_tensor("hT", [D, NP], bf16, kind="ExternalOutput")

    # chunk = DMA/pipeline granularity in columns (multiple of F)
    chunk = F * int(os.environ.get("K_GROUP", "4"))
    psbufs = int(os.environ.get("K_PSBUFS", "2"))
    use_act = int(os.environ.get("K_ACT", "1"))  # alternate DVE/ACT drains
    starts = list(range(0, NP, chunk))
    n_chunks = len(starts)

    with tile.TileContext(nc) as tc:
        with (
            tc.tile_pool(name="const", bufs=1) as cpool,
            tc.tile_pool(name="vg", bufs=n_chunks) as vpool,
            tc.tile_pool(name="hg", bufs=n_chunks) as hpool,
            tc.tile_pool(name="psh", bufs=psbufs, space="PSUM") as pspool,
        ):
            wvT_sb = cpool.tile([D, D], bf16)
            nc.sync.dma_start(out=wvT_sb[:], in_=wvT_in[:])
            b_sb = cpool.tile([D, 1], f32)
            nc.sync.dma_start(out=b_sb[:], in_=b_in[:])

            # prefetch every input chunk before any output trigger is queued
            v_sbs = []
            for g, s0 in enumerate(starts):
                cols = min(chunk, NP - s0)
                v_sb = vpool.tile([D, chunk], bf16, tag="vg")
                nc.sync.dma_start(out=v_sb[:, :cols], in_=vT_in[:, s0 : s0 + cols])
                v_sbs.append(v_sb)

            for g, s0 in enumerate(starts):
                cols = min(chunk, NP - s0)
                v_sb = v_sbs[g]
                h_sb = hpool.tile([D, chunk], bf16, tag="hg")
                h_ps = pspool.tile([P, chunk], f32, tag="hps")
                for f0 in range(0, cols, F):
                    fc = min(F, cols - f0)
                    nc.tensor.matmul(
                        out=h_ps[:, f0 : f0 + fc],
                        lhsT=wvT_sb[:],
                        rhs=v_sb[:, f0 : f0 + fc],
                        start=True,
                        stop=True,
                    )
                if use_act and g % 2 == 1:
                    nc.scalar.add(h_sb[:, :cols], h_ps[:, :cols], b_sb[:])
                else:
                    nc.vector.tensor_scalar_add(
                        h_sb[:, :cols], h_ps[:, :cols], b_sb[:]
                    )
                nc.sync.dma_start(
                    out=hT_out[:, s0 : s0 + cols], in_=h_sb[:, :cols]
                )

    nc.compile()
    return nc


def _get_module(n_tiles):
    if n_tiles not in _module_cache:
        _module_cache[n_tiles] = _build_module(n_tiles)
    return _module_cache[n_tiles]


def kernel(V, E, edge_index, Wq_w, Wq_b, Wk_w, Wk_b, Wv_w, Wv_b, We_w, We_b,
           a_w, a_b, _trace=False):
    V = np.asarray(V, dtype=np.float32)
    n_nodes, d = V.shape
    assert d == D and n_nodes % N_CORES == 0
    npc = n_nodes // N_CORES          # nodes per core
    n_tiles = -(-npc // P)            # 128-col tiles per core
    NP = n_tiles * P

    bf16 = ml_dtypes.bfloat16
    wvT = np.ascontiguousarray(np.asarray(Wv_w, dtype=np.float32).T.astype(bf16))
    bcol = np.ascontiguousarray(np.asarray(Wv_b, dtype=np.float32)[:, None])

    in_maps = []
    for c in range(N_CORES):
        vpT = np.zeros((D, NP), dtype=bf16)
        vpT[:, :npc] = V[c * npc : (c + 1) * npc].astype(bf16).T
        in_maps.append({"vT": vpT, "wvT": wvT, "b": bcol})

    nc = _get_module(n_tiles)
    res = run_bass_kernel_spmd(nc, in_maps, core_ids=list(range(N_CORES)),
                               trace=_trace)
    out = np.concatenate(
        [np.asarray(res.results[c]["hT"])[:, :npc].T.astype(np.float32)
         for c in range(N_CORES)],
        axis=0,
    )

    # nodes with no incoming edge have an empty softmax segment -> H row = 0
    dest = np.asarray(edge_index)[1]
    counts = np.bincount(dest.astype(np.int64), minlength=n_nodes)
    uncovered = np.flatnonzero(counts == 0)
    if uncovered.size:
        out[uncovered] = 0.0

    if _trace:
        return out, res
    return out


# revision 4
# speedup vs baseline: 3.1955x; 1.0265x over previous
"""GATv2 layer kernel for Trainium2 (8 NeuronCores, SPMD).

Math note: in the reference, the per-edge value vectors are gathered from the
*destination* node (Vv = V[dest] @ Wv^T + bv) and the scatter-softmax is also
grouped by destination. Within a destination segment Vv is constant, and the
softmax weights sum to 1, so

    H[n] = (V[n] @ Wv_w^T + Wv_b) * [n has >= 1 incoming edge]

exactly (up to f32 rounding of the softmax-weight sum, ~1e-7 relative).

Device kernel: per-core matmul in transposed layout, h^T = Wv @ v^T. Nodes
are partitioned contiguously across the 8 cores; the small [128,128] weight
is replicated and kept stationary in the PE array while node features stream
through as the moving operand in 512-wide slices (PSUM bank width). Each
chunk's PSUM->SBUF drain fuses the per-partition bias add and the bf16
downcast, split in half across the vector and scalar engines so the two
drains run in parallel and PSUM frees quickly. bf16 I/O halves HBM traffic
and runs the PE at full rate; the rel-err budget (2e-2) dwarfs bf16
rounding (~3e-3).

DMA pacing: concurrent HWDGE transfers share HBM bandwidth round-robin, so
a queue's completion time scales with everything in flight. Chunks are
therefore issued smallest-first (the first chunk lands quickly and compute
starts early) and the last chunk is small again (the final output transfer
+ HBM write receipt is the kernel's tail). v^T/h^T live in DRAM as
[128, nodes] so partition lines are multi-KB and contiguous. All input
triggers are emitted before any output trigger (the sync engine is
in-order; a store's semaphore wait must not stall later loads). Host
stages the transposes/casts and zeroes the (rare) nodes with no incoming
edge after the gather.
"""

import os

import ml_dtypes
import numpy as np

import concourse.bacc as bacc
import concourse.bass as bass
import concourse.mybir as mybir
import concourse.tile as tile
from concourse.bass_utils import run_bass_kernel_spmd

N_CORES = 8
P = 128
D = 128
F = 512       # matmul moving-operand free size = one PSUM bank of f32
CHUNK_MAX = 4  # slices per chunk; 4 banks = half of PSUM

_module_cache = {}


def _chunk_plan(NP):
    """Column counts per pipeline chunk, smallest-first ramp."""
    plan = os.environ.get("K_PLAN", "512,1024,2048")
    cols = [int(x) for x in plan.split(",") if x]
    total = sum(cols)
    assert total <= NP
    while total < NP:
        c = min(CHUNK_MAX * F, NP - total)
        cols.append(c)
        total += c
    return cols


def _build_module(n_tiles):
    """One SPMD NeuronCore program: hT = wvT.T @ vT + b (bf16 I/O).

    n_tiles: 128-column node tiles per core (vT/hT are [128, n_tiles*128]).
    """
    f32 = mybir.dt.float32
    bf16 = mybir.dt.bfloat16
    NP = n_tiles * P

    nc = bacc.Bacc("TRN2", target_bir_lowering=False, debug=False)
    vT_in = nc.dram_tensor("vT", [D, NP], bf16, kind="ExternalInput")
    wvT_in = nc.dram_tensor("wvT", [D, D], bf16, kind="ExternalInput")
    b_in = nc.dram_tensor("b", [D, 1], f32, kind="ExternalInput")
    hT_out = nc.dram_tensor("hT", [D, NP], bf16, kind="ExternalOutput")

    use_act = int(os.environ.get("K_ACT", "1"))  # split drains DVE/ACT
    cols_plan = _chunk_plan(NP)
    n_chunks = len(cols_plan)
    starts = [sum(cols_plan[:g]) for g in range(n_chunks)]

    with tile.TileContext(nc) as tc:
        with (
            tc.tile_pool(name="const", bufs=1) as cpool,
            tc.tile_pool(name="vg", bufs=n_chunks) as vpool,
            tc.tile_pool(name="hg", bufs=n_chunks) as hpool,
            tc.tile_pool(name="psh", bufs=2, space="PSUM") as pspool,
        ):
            # first data chunk before the (small) weight/bias loads: its
            # completion gates the whole compute pipeline
            v_sbs = []
            v_sb = vpool.tile([D, CHUNK_MAX * F], bf16, tag="vg")
            nc.sync.dma_start(
                out=v_sb[:, : cols_plan[0]], in_=vT_in[:, : cols_plan[0]]
            )
            v_sbs.append(v_sb)

            wvT_sb = cpool.tile([D, D], bf16)
            nc.sync.dma_start(out=wvT_sb[:], in_=wvT_in[:])
            b_sb = cpool.tile([D, 1], f32)
            nc.sync.dma_start(out=b_sb[:], in_=b_in[:])

            for g in range(1, n_chunks):
                s0, cols = starts[g], cols_plan[g]
                v_sb = vpool.tile([D, CHUNK_MAX * F], bf16, tag="vg")
                nc.sync.dma_start(out=v_sb[:, :cols], in_=vT_in[:, s0 : s0 + cols])
                v_sbs.append(v_sb)

            for g in range(n_chunks):
                s0, cols = starts[g], cols_plan[g]
                v_sb = v_sbs[g]
                h_sb = hpool.tile([D, CHUNK_MAX * F], bf16, tag="hg")
                h_ps = pspool.tile([P, CHUNK_MAX * F], f32, tag="hps")
                n_sl = -(-cols // F)
                for i in range(n_sl):
                    f0 = i * F
                    fc = min(F, cols - f0)
                    nc.tensor.matmul(
                        out=h_ps[:, f0 : f0 + fc],
                        lhsT=wvT_sb[:],
                        rhs=v_sb[:, f0 : f0 + fc],
                        start=True,
                        stop=True,
                    )
                # parallel half-drains: DVE takes the front slices, ACT the
                # back; both add bias and downcast to bf16
                dcols = min(cols, -(-n_sl // 2) * F) if use_act else cols
                nc.vector.tensor_scalar_add(
                    h_sb[:, :dcols], h_ps[:, :dcols], b_sb[:]
                )
                if dcols < cols:
                    nc.scalar.add(
                        h_sb[:, dcols:cols], h_ps[:, dcols:cols], b_sb[:]
                    )
                nc.sync.dma_start(
                    out=hT_out[:, s0 : s0 + cols], in_=h_sb[:, :cols]
                )

    nc.compile()
    return nc


def _get_module(n_tiles):
    if n_tiles not in _module_cache:
        _module_cache[n_tiles] = _build_module(n_tiles)
    return _module_cache[n_tiles]


def kernel(V, E, edge_index, Wq_w, Wq_b, Wk_w, Wk_b, Wv_w, Wv_b, We_w, We_b,
           a_w, a_b, _trace=False):
    V = np.asarray(V, dtype=np.float32)
    n_nodes, d = V.shape
    assert d == D and n_nodes % N_CORES == 0
    npc = n_nodes // N_CORES          # nodes per core
    n_tiles = -(-npc // P)            # 128-col tiles per core
    NP = n_tiles * P

    bf16 = ml_dtypes.bfloat16
    wvT = np.ascontiguousarray(np.asarray(Wv_w, dtype=np.float32).T.astype(bf16))
    bcol = np.ascontiguousarray(np.asarray(Wv_b, dtype=np.float32)[:, None])

    in_maps = []
    for c in range(N_CORES):
        vpT = np.zeros((D, NP), dtype=bf16)
        vpT[:, :npc] = V[c * npc : (c + 1) * npc].astype(bf16).T
        in_maps.append({"vT": vpT, "wvT": wvT, "b": bcol})

    nc = _get_module(n_tiles)
    res = run_bass_kernel_spmd(nc, in_maps, core_ids=list(range(N_CORES)),
                               trace=_trace)
    out = np.concatenate(
        [np.asarray(res.results[c]["hT"])[:, :npc].T.astype(np.float32)
         for c in range(N_CORES)],
        axis=0,
    )

    # nodes with no incoming edge have an empty softmax segment -> H row = 0
    dest = np.asarray(edge_index)[1]
    counts = np.bincount(dest.astype(np.int64), minlength=n_nodes)
    uncovered = np.flatnonzero(counts == 0)
    if uncovered.size:
        out[uncovered] = 0.0

    if _trace:
        return out, res
    return out
